# revision 23
# baseline (speedup 1.0000x reference)
"""Trainium2 Bass kernel for MBDCompressor3D.

Computes, for Q=262144 query points against two sets of 64 3D Gaussians:
  phi = normalized gaussian weights (coeff set), psi = (basis set)
  moving_coeff = phi @ C            [Q, 2]
  moving_basis = psi @ B            [Q, 2, 27]
  mbd          = sum_l mc_l * mb_l  [Q, 27]
  rec          = MLP([mbd, p])      [Q, 27]

Strategy: the Mahalanobis forms reduce to one [Q,10]@[10,128] matmul over a
host-precomputed quadratic-feature map. Data-parallel over Q across the 8
NeuronCores (SPMD, no collectives); tiny parameters are replicated.
"""

import numpy as np

import concourse.bass as bass
import concourse.mybir as mybir
import concourse.tile as tile
from concourse.bass_utils import run_bass_kernel_spmd

# Problem constants (hardcoded per harness contract).
Q = 262144
MG = 64          # coeff gaussians
NG = 64          # basis gaussians
KG = MG + NG     # 128 combined
L = 2
D = 27
H = 32
EPS = 1e-8

NCORES = 8
QC = Q // NCORES     # 32768 per core
QT = 512             # queries per device tile
NSUB = QT // 128     # 4 subtiles of 128 queries
NT = QC // QT        # 64 tiles per core

# Packed per-query output columns: [rec 27 | mc 2 | mb 54 | mbd 27] = 110,
# plus 3 scratch columns of coords appended for the on-chip transpose input.
C_REC, C_MC, C_MB, C_MBD, C_CO = 0, 27, 29, 83, 110
C_OUT = 110
C_ALL = 113

F32 = mybir.dt.float32
F32R = mybir.dt.float32r

LAST_EXEC_NS = None
_LAST_IN_MAPS = None


# --------------------------------------------------------------------------
# Host-side math (float64 for the tiny parameter transforms)
# --------------------------------------------------------------------------

def _quat_to_rot(q):
    q = q / (np.linalg.norm(q, axis=-1, keepdims=True) + EPS)
    w, x, y, z = q[..., 0], q[..., 1], q[..., 2], q[..., 3]
    r0 = np.stack([1 - 2 * y * y - 2 * z * z, 2 * x * y - 2 * w * z, 2 * x * z + 2 * w * y], axis=-1)
    r1 = np.stack([2 * x * y + 2 * w * z, 1 - 2 * x * x - 2 * z * z, 2 * y * z - 2 * w * x], axis=-1)
    r2 = np.stack([2 * x * z - 2 * w * y, 2 * y * z + 2 * w * x, 1 - 2 * x * x - 2 * y * y], axis=-1)
    return np.stack([r0, r1, r2], axis=-2)


def _quad_coeffs(mu, log_s, quat):
    """[K] gaussians -> A [10, K]: mah(p) = F(p) . A[:, k] with
    F = [x^2, y^2, z^2, xy, xz, yz, x, y, z, 1]."""
    mu = mu.astype(np.float64)
    s = np.exp(log_s.astype(np.float64))
    R = _quat_to_rot(quat.astype(np.float64))
    sinv = 1.0 / (s * s + EPS)                       # [K, 3]
    prec = np.einsum('kij,kj,klj->kil', R, sinv, R)  # [K, 3, 3]
    pmu = np.einsum('kil,kl->ki', prec, mu)          # [K, 3]
    K = mu.shape[0]
    A = np.empty((10, K), np.float64)
    A[0] = prec[:, 0, 0]
    A[1] = prec[:, 1, 1]
    A[2] = prec[:, 2, 2]
    A[3] = 2 * prec[:, 0, 1]
    A[4] = 2 * prec[:, 0, 2]
    A[5] = 2 * prec[:, 1, 2]
    A[6] = -2 * pmu[:, 0]
    A[7] = -2 * pmu[:, 1]
    A[8] = -2 * pmu[:, 2]
    A[9] = np.einsum('ki,ki->k', pmu, mu)
    return A


def _features_t(p):
    """coords [n, 3] f32 -> F^T [10, n] f32."""
    x, y, z = p[:, 0], p[:, 1], p[:, 2]
    return np.stack([x * x, y * y, z * z, x * y, x * z, y * z,
                     x, y, z, np.ones_like(x)], axis=0).astype(np.float32)


def _round_f32r(x):
    """Round f32 to the PE's FP32R format (11-bit mantissa, RNE-ish)."""
    b = np.asarray(x, np.float32).view(np.uint32)
    r = (b + 0x800 + ((b >> 12) & 1)) & np.uint32(0xFFFFF000)
    out = r.view(np.float32).copy()
    out[~np.isfinite(np.asarray(x, np.float32))] = np.asarray(
        x, np.float32)[~np.isfinite(np.asarray(x, np.float32))]
    return out


def _split_f32r(x):
    """x ~= hi + lo with both exactly representable in FP32R."""
    x = np.asarray(x, np.float32)
    hi = _round_f32r(x)
    lo = _round_f32r((x.astype(np.float64) - hi.astype(np.float64)).astype(np.float32))
    return hi, lo


# --------------------------------------------------------------------------
# Walrus in this toolchain rejects instructions with >2 semaphore waits; the
# TileContext exit drain (and occasionally a scheduled instruction) can carry
# more. Post-pass: hoist excess waits onto same-engine NOPs placed just
# before the offending instruction.
# --------------------------------------------------------------------------

_MAXW = 1
_splitctr = [0]


def _split_excess_waits(nc):
    for f in nc.m.functions:
        for bb in f.blocks:
            insts = list(bb.instructions)
            out = []
            changed = False
            for ins in insts:
                si = ins.sync_info
                waits = list(si.on_wait) if (si is not None and si.on_wait) else []
                maxw = 1 if "DMA" in type(ins).__name__ else _MAXW
                if len(waits) > maxw:
                    changed = True
                    extra, keep = waits[:-maxw], waits[-maxw:]
                    for i in range(0, len(extra), _MAXW):
                        _splitctr[0] += 1
                        nop = mybir.InstNoOp(
                            name=f"ws-{_splitctr[0]}",
                            engine=ins.engine,
                            ins=[],
                            outs=[],
                            sync_info=type(si)(
                                on_wait=list(extra[i:i + _MAXW]), on_update=[]
                            ),
                        )
                        nc.register_instruction(nop, overwrite=True)
                        out.append(nop)
                    si.on_wait = keep
                out.append(ins)
            if changed:
                bb.instructions[:] = out


class SplitDrainTileContext(tile.TileContext):
    """Tail drain emitted with one wait per outstanding sem; split them."""

    def _drain_and_barrier(self, tick_clock, wait_clock):
        import bass_rust as _br
        drain_inst = self.nc.sync.drain()
        wait_clock.add_sem_waits(
            drain_inst.ins, _br.ScopedClock({None: tick_clock.global_clock})
        )
        si = drain_inst.ins.sync_info
        waits = list(si.on_wait or []) if si is not None else []
        if len(waits) > 1:
            si.on_wait = waits[:1]
            for w in waits[1:]:
                nop = self.nc.sync.nop()
                nsi = nop.ins.sync_info
                if nsi is None:
                    nop.ins.sync_info = type(si)(on_wait=[w], on_update=[])
                else:
                    nsi.on_wait = [w]
        self.nc.all_engine_barrier()
        assert self.sems is not None
        popped = self.nc._tile_sem_poison_stack.pop()
        assert popped is self._sem_poison
        self.nc.clear_and_free_semaphores(list(self.sems.allocated().values()))
        self.nc.all_engine_barrier()


# --------------------------------------------------------------------------
# Device program
# --------------------------------------------------------------------------

def _bcast(ap, n):
    """Broadcast the (size-1) innermost free dim of `ap` to size n."""
    a = list(ap.ap)
    assert a[-1][1] == 1, a
    a[-1] = [0, n]
    return bass.AP(tensor=ap.tensor, offset=ap.offset, ap=a)


def _r(ap):
    return ap.bitcast(F32R)


NF = 19  # 10 hi-feature rows + 9 lo-feature rows (ones row has no lo)
import os as _os
USE_GPSIMD_MBD = bool(int(_os.environ.get("K_GPSIMD_MBD", "0")))
# Bisection stages: 4=full, 1=DMA only, 2=+mm1/exp, 3=+mm2'/epilogue (no MLP)
K_STAGE = int(_os.environ.get("K_STAGE", "4"))
K_LOADENG = _os.environ.get("K_LOADENG", "scalar")  # sync|scalar
K_BUFS = int(_os.environ.get("K_BUFS", "2"))


def _build_program(repeat=1):
    nc = bass.Bass()

    ft_d = nc.dram_tensor("ft", [NF, QC], F32R, kind="ExternalInput")
    co_d = nc.dram_tensor("co", [NT, 128, NSUB, 3], F32, kind="ExternalInput")
    a1_d = nc.dram_tensor("a1", [NF, KG], F32R, kind="ExternalInput")
    a2_d = nc.dram_tensor("a2", [10, KG], F32R, kind="ExternalInput")
    cb_d = nc.dram_tensor("cb", [KG, 58], F32, kind="ExternalInput")
    w1_d = nc.dram_tensor("w1", [D + 3, H], F32, kind="ExternalInput")
    w2_d = nc.dram_tensor("w2", [H, D], F32, kind="ExternalInput")
    b1_d = nc.dram_tensor("b1", [H, 1], F32, kind="ExternalInput")
    b2_d = nc.dram_tensor("b2b", [128, D], F32, kind="ExternalInput")
    id_d = nc.dram_tensor("ident", [128, 128], F32, kind="ExternalInput")
    out_d = nc.dram_tensor("out", [QC, C_OUT], F32, kind="ExternalOutput")

    with SplitDrainTileContext(nc) as tc:
        with (
            tc.tile_pool(name="singles", bufs=1) as singles,
            tc.tile_pool(name="ftp", bufs=3) as ftp,
            tc.tile_pool(name="gp", bufs=2) as gp,
            tc.tile_pool(name="small", bufs=3) as small,
            tc.tile_pool(name="mlp", bufs=3) as mlp,
            tc.tile_pool(name="outp", bufs=8) as outp,
            tc.tile_pool(name="ps_mah", bufs=1, space="PSUM") as ps_mah,
            tc.tile_pool(name="ps_ut", bufs=2, space="PSUM") as ps_ut,
            tc.tile_pool(name="ps_mlp", bufs=1, space="PSUM") as ps_mlp,
            tc.tile_pool(name="ps_h", bufs=1, space="PSUM") as ps_h,
            tc.tile_pool(name="ps_rec", bufs=2, space="PSUM") as ps_rec,
        ):
            a1_sb = singles.tile([NF, KG], F32R)
            nc.sync.dma_start(out=a1_sb[:], in_=a1_d[:])
            a2_sb = singles.tile([10, KG], F32R)
            nc.sync.dma_start(out=a2_sb[:], in_=a2_d[:])
            cb_sb = singles.tile([KG, 58], F32)
            nc.sync.dma_start(out=cb_sb[:], in_=cb_d[:])
            w1_sb = singles.tile([D + 3, H], F32)
            nc.sync.dma_start(out=w1_sb[:], in_=w1_d[:])
            w2_sb = singles.tile([H, D], F32)
            nc.sync.dma_start(out=w2_sb[:], in_=w2_d[:])
            b1_sb = singles.tile([H, 1], F32)
            nc.sync.dma_start(out=b1_sb[:], in_=b1_d[:])
            b2_sb = singles.tile([128, D], F32)
            nc.sync.dma_start(out=b2_sb[:], in_=b2_d[:])
            id_sb = singles.tile([128, 128], F32)
            nc.sync.dma_start(out=id_sb[:], in_=id_d[:])

            import contextlib
            rep_ctx = (tc.For_i(0, repeat, 1) if repeat > 1
                       else contextlib.nullcontext())

            state = {}  # per-tile live tiles

            def p0_load(it):
                o = it * QT
                load_eng = nc.sync if K_LOADENG == "sync" else nc.scalar
                ft_t = ftp.tile([NF, QT], F32R)
                load_eng.dma_start(out=ft_t[:], in_=ft_d[:, o:o + QT])
                out_sb = outp.tile([128, NSUB, C_ALL], F32)
                load_eng.dma_start(out=out_sb[:, :, C_CO:C_ALL], in_=co_d[it])
                state[it] = {"ft": ft_t, "out": out_sb}

            def p1_front(it):
                st = state[it]
                ft_t = st["ft"]
                mah_ps = ps_mah.tile([128, QT], F32)
                nc.tensor.matmul(mah_ps[:], a1_sb[:], ft_t[:],
                                 start=True, stop=False)
                nc.tensor.matmul(mah_ps[:], a2_sb[:], ft_t[0:10, :],
                                 start=False, stop=True)
                g_sb = gp.tile([128, QT], F32)
                nc.scalar.activation(out=g_sb[:], in_=mah_ps[:],
                                     func=mybir.ActivationFunctionType.Exp,
                                     scale=-0.5)
                ut_ps = ps_ut.tile([128, NSUB, 58], F32)
                for s in range(NSUB):
                    nc.tensor.matmul(ut_ps[:, s, :],
                                     g_sb[:, s * 128:(s + 1) * 128],
                                     cb_sb[:], start=True, stop=True)
                st["ut"] = ut_ps

            def p2_epilogue(it):
                st = state[it]
                ut_ps = st["ut"]
                out_sb = st["out"]
                se = small.tile([128, NSUB, 2], F32)
                nc.vector.tensor_scalar_add(out=se[:], in0=ut_ps[:, :, 0:2],
                                            scalar1=EPS)
                rcp = small.tile([128, NSUB, 2], F32)
                nc.vector.reciprocal(out=rcp[:], in_=se[:])
                nc.vector.tensor_mul(out_sb[:, :, C_MC:C_MC + 2],
                                     ut_ps[:, :, 2:4],
                                     _bcast(rcp[:, :, 0:1], 2))
                nc.vector.tensor_mul(out_sb[:, :, C_MB:C_MB + 54],
                                     ut_ps[:, :, 4:58],
                                     _bcast(rcp[:, :, 1:2], 54))
                _eng_t = nc.gpsimd if USE_GPSIMD_MBD else nc.vector
                t2 = small.tile([128, NSUB, D], F32)
                _eng_t.tensor_mul(t2[:],
                                  out_sb[:, :, C_MB + D:C_MB + 2 * D],
                                  _bcast(out_sb[:, :, C_MC + 1:C_MC + 2], D))
                t1 = small.tile([128, NSUB, D], F32)
                _eng_t.tensor_mul(t1[:],
                                  out_sb[:, :, C_MB:C_MB + D],
                                  _bcast(out_sb[:, :, C_MC:C_MC + 1], D))
                _eng_t.tensor_add(out_sb[:, :, C_MBD:C_MBD + D], t1[:], t2[:])

            def p3_transpose(it):
                st = state[it]
                out_sb = st["out"]
                mlpt_ps = ps_mlp.tile([D + 3, QT], F32)
                for s in range(NSUB):
                    nc.tensor.transpose(mlpt_ps[:, s * 128:(s + 1) * 128],
                                        out_sb[:, s, C_MBD:C_ALL],
                                        id_sb[:])
                st["mlpt_ps"] = mlpt_ps

            def p4_copy(it):
                st = state[it]
                mlpt_ps = st["mlpt_ps"]
                mlpt_sb = mlp.tile([D + 3, QT], F32)
                nc.vector.tensor_copy(mlpt_sb[:, 0:QT // 2],
                                      mlpt_ps[:, 0:QT // 2])
                nc.scalar.copy(mlpt_sb[:, QT // 2:QT],
                               mlpt_ps[:, QT // 2:QT])
                st["mlpt_sb"] = mlpt_sb

            def p5_mm3(it):
                st = state[it]
                h_ps = ps_h.tile([H, QT], F32)
                nc.tensor.matmul(h_ps[:], w1_sb[:], st["mlpt_sb"][:],
                                 start=True, stop=True)
                st["h_ps"] = h_ps

            def p6_relu(it):
                st = state[it]
                h_ps = st["h_ps"]
                h_sb = mlp.tile([H, QT], F32)
                nc.vector.tensor_scalar(out=h_sb[:, 0:QT // 2],
                                        in0=h_ps[:, 0:QT // 2],
                                        scalar1=b1_sb[:],
                                        scalar2=0.0,
                                        op0=mybir.AluOpType.add,
                                        op1=mybir.AluOpType.max)
                nc.scalar.activation(out=h_sb[:, QT // 2:QT],
                                     in_=h_ps[:, QT // 2:QT],
                                     func=mybir.ActivationFunctionType.Relu,
                                     bias=b1_sb[:], scale=1.0)
                st["h_sb"] = h_sb

            def p7_mm4(it):
                st = state[it]
                h_sb = st["h_sb"]
                rec_ps = ps_rec.tile([128, NSUB, D], F32)
                for s in range(NSUB):
                    nc.tensor.matmul(rec_ps[:, s, :],
                                     h_sb[:, s * 128:(s + 1) * 128],
                                     w2_sb[:], start=True, stop=True)
                st["rec_ps"] = rec_ps

            def p8_store(it):
                st = state.pop(it)
                out_sb = st["out"]
                b2b = bass.AP(tensor=b2_sb.tensor, offset=b2_sb.offset,
                              ap=[b2_sb.ap[0], [0, NSUB], b2_sb.ap[1]])
                nc.vector.tensor_add(out_sb[:, :, C_REC:C_REC + D],
                                     st["rec_ps"][:], b2b)
                o = it * QT
                dst = out_d[o:o + QT].rearrange("(s p) d -> p s d", p=128)
                nc.sync.dma_start(out=dst, in_=out_sb[:, :, 0:C_OUT])

            if K_STAGE >= 4:
                phases = [(0, p0_load), (1, p1_front), (2, p2_epilogue),
                          (3, p3_transpose), (3, p4_copy), (4, p5_mm3),
                          (4, p6_relu), (5, p7_mm4), (6, p8_store)]
            elif K_STAGE == 1:
                def p_dummy(it):
                    st = state[it]
                    nc.vector.memset(st["out"][:, :, 0:C_OUT], 0.0)

                def p_store1(it):
                    st = state.pop(it)
                    o = it * QT
                    dst = out_d[o:o + QT].rearrange("(s p) d -> p s d", p=128)
                    nc.sync.dma_start(out=dst, in_=st["out"][:, :, 0:C_OUT])
                phases = [(0, p0_load), (1, p_dummy), (2, p_store1)]
            else:
                raise ValueError("K_STAGE 2/3 removed in pipelined version")

            max_off = max(off for off, _ in phases)
            with rep_ctx:
                for step in range(NT + max_off):
                    for off, fn in phases:
                        it = step - off
                        if 0 <= it < NT:
                            fn(it)

    _split_excess_waits(nc)
    return nc


_PROGRAM = None


def _program():
    global _PROGRAM
    if _PROGRAM is None:
        _PROGRAM = _build_program()
    return _PROGRAM


# --------------------------------------------------------------------------
# Entry point
# --------------------------------------------------------------------------

def kernel(coords, coeff_mu, coeff_log_s, coeff_q, basis_mu, basis_log_s,
           basis_q, C, B, W1, b1, W2, b2):
    global LAST_EXEC_NS
    import os

    coords = np.asarray(coords, np.float32)

    A = np.concatenate([
        _quad_coeffs(np.asarray(coeff_mu), np.asarray(coeff_log_s), np.asarray(coeff_q)),
        _quad_coeffs(np.asarray(basis_mu), np.asarray(basis_log_s), np.asarray(basis_q)),
    ], axis=1).astype(np.float32)                    # [10, 128]
    a_hi, a_lo = _split_f32r(A)
    # mm1a pairs A_hi with [F_hi; F_lo]; mm1b pairs A_lo with F_hi.
    a1 = np.concatenate([a_hi, a_hi[0:9]], axis=0)   # [19, 128]
    a2 = a_lo                                        # [10, 128]

    CB = np.zeros((KG, 58), np.float32)
    CB[:MG, 0] = 1.0
    CB[MG:, 1] = 1.0
    CB[:MG, 2:4] = np.asarray(C, np.float32)
    CB[MG:, 4:58] = np.asarray(B, np.float32).reshape(NG, L * D)

    w1 = np.asarray(W1, np.float32)                  # [30, 32]
    w2 = np.asarray(W2, np.float32)                  # [32, 27]
    b1v = np.asarray(b1, np.float32).reshape(H, 1)
    b2b = np.broadcast_to(np.asarray(b2, np.float32).reshape(1, D), (128, D)).copy()
    ident = np.eye(128, dtype=np.float32)

    in_maps = []
    for c in range(NCORES):
        shard = coords[c * QC:(c + 1) * QC]
        f = _features_t(shard)                                             # [10, QC]
        f_hi, f_lo = _split_f32r(f)
        ft = np.ascontiguousarray(np.concatenate([f_hi, f_lo[0:9]], axis=0))  # [19, QC]
        co = np.ascontiguousarray(
            shard.reshape(NT, NSUB, 128, 3).transpose(0, 2, 1, 3))         # [NT,128,4,3]
        in_maps.append({
            "ft": ft, "co": co, "a1": a1, "a2": a2, "cb": CB, "w1": w1,
            "w2": w2, "b1": b1v, "b2b": b2b, "ident": ident,
        })

    global _LAST_IN_MAPS
    _LAST_IN_MAPS = in_maps
    nc = _program()
    trace = bool(int(os.environ.get("KERNEL_TRACE", "0")))
    res = run_bass_kernel_spmd(nc, in_maps, core_ids=list(range(NCORES)),
                               trace=trace)
    LAST_EXEC_NS = res.exec_time_ns

    out = np.concatenate([res.results[c]["out"] for c in range(NCORES)], axis=0)
    rec = np.ascontiguousarray(out[:, C_REC:C_REC + D])
    mc = np.ascontiguousarray(out[:, C_MC:C_MC + 2])
    mb = np.ascontiguousarray(out[:, C_MB:C_MB + 54]).reshape(Q, L, D)
    mbd = np.ascontiguousarray(out[:, C_MBD:C_MBD + D])
    return rec, mc, mb, mbd


# revision 26
# speedup vs baseline: 1.8428x; 1.8428x over previous
"""Trainium2 Bass kernel for MBDCompressor3D.

Computes, for Q=262144 query points against two sets of 64 3D Gaussians:
  phi = normalized gaussian weights (coeff set), psi = (basis set)
  moving_coeff = phi @ C            [Q, 2]
  moving_basis = psi @ B            [Q, 2, 27]
  mbd          = sum_l mc_l * mb_l  [Q, 27]
  rec          = MLP([mbd, p])      [Q, 27]

Strategy: the Mahalanobis forms reduce to one [Q,10]@[10,128] matmul over a
host-precomputed quadratic-feature map. Data-parallel over Q across the 8
NeuronCores (SPMD, no collectives); tiny parameters are replicated.
"""

import numpy as np

import concourse.bass as bass
import concourse.mybir as mybir
import concourse.tile as tile
from concourse.bass_utils import run_bass_kernel_spmd

# Problem constants (hardcoded per harness contract).
Q = 262144
MG = 64          # coeff gaussians
NG = 64          # basis gaussians
KG = MG + NG     # 128 combined
L = 2
D = 27
H = 32
EPS = 1e-8

NCORES = 8
QC = Q // NCORES     # 32768 per core
QT = 512             # queries per device tile
NSUB = QT // 128     # 4 subtiles of 128 queries
NT = QC // QT        # 64 tiles per core

# Packed per-query output columns: [rec 27 | mc 2 | mb 54 | mbd 27] = 110,
# plus 3 scratch columns of coords appended for the on-chip transpose input.
C_REC, C_MC, C_MB, C_MBD, C_CO = 0, 27, 29, 83, 110
C_OUT = 110
C_ALL = 113

F32 = mybir.dt.float32
F32R = mybir.dt.float32r

LAST_EXEC_NS = None
_LAST_IN_MAPS = None


# --------------------------------------------------------------------------
# Host-side math (float64 for the tiny parameter transforms)
# --------------------------------------------------------------------------

def _quat_to_rot(q):
    q = q / (np.linalg.norm(q, axis=-1, keepdims=True) + EPS)
    w, x, y, z = q[..., 0], q[..., 1], q[..., 2], q[..., 3]
    r0 = np.stack([1 - 2 * y * y - 2 * z * z, 2 * x * y - 2 * w * z, 2 * x * z + 2 * w * y], axis=-1)
    r1 = np.stack([2 * x * y + 2 * w * z, 1 - 2 * x * x - 2 * z * z, 2 * y * z - 2 * w * x], axis=-1)
    r2 = np.stack([2 * x * z - 2 * w * y, 2 * y * z + 2 * w * x, 1 - 2 * x * x - 2 * y * y], axis=-1)
    return np.stack([r0, r1, r2], axis=-2)


def _quad_coeffs(mu, log_s, quat):
    """[K] gaussians -> A [10, K]: mah(p) = F(p) . A[:, k] with
    F = [x^2, y^2, z^2, xy, xz, yz, x, y, z, 1]."""
    mu = mu.astype(np.float64)
    s = np.exp(log_s.astype(np.float64))
    R = _quat_to_rot(quat.astype(np.float64))
    sinv = 1.0 / (s * s + EPS)                       # [K, 3]
    prec = np.einsum('kij,kj,klj->kil', R, sinv, R)  # [K, 3, 3]
    pmu = np.einsum('kil,kl->ki', prec, mu)          # [K, 3]
    K = mu.shape[0]
    A = np.empty((10, K), np.float64)
    A[0] = prec[:, 0, 0]
    A[1] = prec[:, 1, 1]
    A[2] = prec[:, 2, 2]
    A[3] = 2 * prec[:, 0, 1]
    A[4] = 2 * prec[:, 0, 2]
    A[5] = 2 * prec[:, 1, 2]
    A[6] = -2 * pmu[:, 0]
    A[7] = -2 * pmu[:, 1]
    A[8] = -2 * pmu[:, 2]
    A[9] = np.einsum('ki,ki->k', pmu, mu)
    return A


def _features_t(p):
    """coords [n, 3] f32 -> F^T [10, n] f32."""
    x, y, z = p[:, 0], p[:, 1], p[:, 2]
    return np.stack([x * x, y * y, z * z, x * y, x * z, y * z,
                     x, y, z, np.ones_like(x)], axis=0).astype(np.float32)


def _round_f32r(x):
    """Round f32 to the PE's FP32R format (11-bit mantissa, RNE-ish)."""
    b = np.asarray(x, np.float32).view(np.uint32)
    r = (b + 0x800 + ((b >> 12) & 1)) & np.uint32(0xFFFFF000)
    out = r.view(np.float32).copy()
    out[~np.isfinite(np.asarray(x, np.float32))] = np.asarray(
        x, np.float32)[~np.isfinite(np.asarray(x, np.float32))]
    return out


def _split_f32r(x):
    """x ~= hi + lo with both exactly representable in FP32R."""
    x = np.asarray(x, np.float32)
    hi = _round_f32r(x)
    lo = _round_f32r((x.astype(np.float64) - hi.astype(np.float64)).astype(np.float32))
    return hi, lo


# --------------------------------------------------------------------------
# Walrus in this toolchain rejects instructions with >2 semaphore waits; the
# TileContext exit drain (and occasionally a scheduled instruction) can carry
# more. Post-pass: hoist excess waits onto same-engine NOPs placed just
# before the offending instruction.
# --------------------------------------------------------------------------

_MAXW = 1
_splitctr = [0]


def _split_excess_waits(nc):
    for f in nc.m.functions:
        for bb in f.blocks:
            insts = list(bb.instructions)
            out = []
            changed = False
            for ins in insts:
                si = ins.sync_info
                waits = list(si.on_wait) if (si is not None and si.on_wait) else []
                maxw = 1 if "DMA" in type(ins).__name__ else _MAXW
                if len(waits) > maxw:
                    changed = True
                    extra, keep = waits[:-maxw], waits[-maxw:]
                    for i in range(0, len(extra), _MAXW):
                        _splitctr[0] += 1
                        nop = mybir.InstNoOp(
                            name=f"ws-{_splitctr[0]}",
                            engine=ins.engine,
                            ins=[],
                            outs=[],
                            sync_info=type(si)(
                                on_wait=list(extra[i:i + _MAXW]), on_update=[]
                            ),
                        )
                        nc.register_instruction(nop, overwrite=True)
                        out.append(nop)
                    si.on_wait = keep
                out.append(ins)
            if changed:
                bb.instructions[:] = out


class SplitDrainTileContext(tile.TileContext):
    """Tail drain emitted with one wait per outstanding sem; split them."""

    def _drain_and_barrier(self, tick_clock, wait_clock):
        import bass_rust as _br
        drain_inst = self.nc.sync.drain()
        wait_clock.add_sem_waits(
            drain_inst.ins, _br.ScopedClock({None: tick_clock.global_clock})
        )
        si = drain_inst.ins.sync_info
        waits = list(si.on_wait or []) if si is not None else []
        if len(waits) > 1:
            si.on_wait = waits[:1]
            for w in waits[1:]:
                nop = self.nc.sync.nop()
                nsi = nop.ins.sync_info
                if nsi is None:
                    nop.ins.sync_info = type(si)(on_wait=[w], on_update=[])
                else:
                    nsi.on_wait = [w]
        self.nc.all_engine_barrier()
        assert self.sems is not None
        popped = self.nc._tile_sem_poison_stack.pop()
        assert popped is self._sem_poison
        self.nc.clear_and_free_semaphores(list(self.sems.allocated().values()))
        self.nc.all_engine_barrier()


# --------------------------------------------------------------------------
# Device program
# --------------------------------------------------------------------------

def _bcast(ap, n):
    """Broadcast the (size-1) innermost free dim of `ap` to size n."""
    a = list(ap.ap)
    assert a[-1][1] == 1, a
    a[-1] = [0, n]
    return bass.AP(tensor=ap.tensor, offset=ap.offset, ap=a)


def _r(ap):
    return ap.bitcast(F32R)


NF = 19  # 10 hi-feature rows + 9 lo-feature rows (ones row has no lo)
import os as _os
USE_GPSIMD_MBD = bool(int(_os.environ.get("K_GPSIMD_MBD", "0")))
# Bisection stages: 4=full, 1=DMA only, 2=+mm1/exp, 3=+mm2'/epilogue (no MLP)
K_STAGE = int(_os.environ.get("K_STAGE", "4"))
K_LOADENG = _os.environ.get("K_LOADENG", "scalar")  # sync|scalar
K_BUFS = int(_os.environ.get("K_BUFS", "2"))


def _build_program(repeat=1):
    nc = bass.Bass()

    ft_d = nc.dram_tensor("ft", [NF, QC], F32R, kind="ExternalInput")
    co_d = nc.dram_tensor("co", [NT, 128, NSUB, 3], F32, kind="ExternalInput")
    a1_d = nc.dram_tensor("a1", [NF, KG], F32R, kind="ExternalInput")
    a2_d = nc.dram_tensor("a2", [10, KG], F32R, kind="ExternalInput")
    cb_d = nc.dram_tensor("cb", [KG, 58], F32, kind="ExternalInput")
    w1_d = nc.dram_tensor("w1", [D + 3, H], F32, kind="ExternalInput")
    w2_d = nc.dram_tensor("w2", [H, D], F32, kind="ExternalInput")
    b1_d = nc.dram_tensor("b1", [H, 1], F32, kind="ExternalInput")
    b2_d = nc.dram_tensor("b2b", [128, D], F32, kind="ExternalInput")
    id_d = nc.dram_tensor("ident", [128, 128], F32, kind="ExternalInput")
    out_d = nc.dram_tensor("out", [QC, C_OUT], F32, kind="ExternalOutput")

    with SplitDrainTileContext(nc) as tc:
        with (
            tc.tile_pool(name="singles", bufs=1) as singles,
            tc.tile_pool(name="ftp", bufs=3) as ftp,
            tc.tile_pool(name="gp", bufs=2) as gp,
            tc.tile_pool(name="small", bufs=3) as small,
            tc.tile_pool(name="mlp", bufs=3) as mlp,
            tc.tile_pool(name="outp", bufs=8) as outp,
            tc.tile_pool(name="ps_mah", bufs=1, space="PSUM") as ps_mah,
            tc.tile_pool(name="ps_ut", bufs=2, space="PSUM") as ps_ut,
            tc.tile_pool(name="ps_mlp", bufs=1, space="PSUM") as ps_mlp,
            tc.tile_pool(name="ps_h", bufs=1, space="PSUM") as ps_h,
            tc.tile_pool(name="ps_rec", bufs=2, space="PSUM") as ps_rec,
        ):
            a1_sb = singles.tile([NF, KG], F32R)
            nc.sync.dma_start(out=a1_sb[:], in_=a1_d[:])
            a2_sb = singles.tile([10, KG], F32R)
            nc.sync.dma_start(out=a2_sb[:], in_=a2_d[:])
            cb_sb = singles.tile([KG, 58], F32)
            nc.sync.dma_start(out=cb_sb[:], in_=cb_d[:])
            w1_sb = singles.tile([D + 3, H], F32)
            nc.sync.dma_start(out=w1_sb[:], in_=w1_d[:])
            w2_sb = singles.tile([H, D], F32)
            nc.sync.dma_start(out=w2_sb[:], in_=w2_d[:])
            b1_sb = singles.tile([H, 1], F32)
            nc.sync.dma_start(out=b1_sb[:], in_=b1_d[:])
            b2_sb = singles.tile([128, D], F32)
            nc.sync.dma_start(out=b2_sb[:], in_=b2_d[:])
            id_sb = singles.tile([128, 128], F32)
            nc.sync.dma_start(out=id_sb[:], in_=id_d[:])

            import contextlib
            rep_ctx = (tc.For_i(0, repeat, 1) if repeat > 1
                       else contextlib.nullcontext())

            state = {}  # per-tile live tiles

            def p0_load(it):
                o = it * QT
                load_eng = nc.sync if K_LOADENG == "sync" else nc.scalar
                ft_t = ftp.tile([NF, QT], F32R)
                load_eng.dma_start(out=ft_t[:], in_=ft_d[:, o:o + QT])
                out_sb = outp.tile([128, NSUB, C_ALL], F32)
                load_eng.dma_start(out=out_sb[:, :, C_CO:C_ALL], in_=co_d[it])
                state[it] = {"ft": ft_t, "out": out_sb}

            def p1_front(it):
                st = state[it]
                ft_t = st["ft"]
                mah_ps = ps_mah.tile([128, QT], F32)
                nc.tensor.matmul(mah_ps[:], a1_sb[:], ft_t[:],
                                 start=True, stop=False)
                nc.tensor.matmul(mah_ps[:], a2_sb[:], ft_t[0:10, :],
                                 start=False, stop=True)
                g_sb = gp.tile([128, QT], F32)
                nc.scalar.activation(out=g_sb[:], in_=mah_ps[:],
                                     func=mybir.ActivationFunctionType.Exp,
                                     scale=-0.5)
                ut_ps = ps_ut.tile([128, NSUB, 58], F32)
                for s in range(NSUB):
                    nc.tensor.matmul(ut_ps[:, s, :],
                                     g_sb[:, s * 128:(s + 1) * 128],
                                     cb_sb[:], start=True, stop=True)
                st["ut"] = ut_ps

            def p2_epilogue(it):
                st = state[it]
                ut_ps = st["ut"]
                out_sb = st["out"]
                se = small.tile([128, NSUB, 2], F32)
                nc.vector.tensor_scalar_add(out=se[:], in0=ut_ps[:, :, 0:2],
                                            scalar1=EPS)
                rcp = small.tile([128, NSUB, 2], F32)
                nc.vector.reciprocal(out=rcp[:], in_=se[:])
                nc.vector.tensor_mul(out_sb[:, :, C_MC:C_MC + 2],
                                     ut_ps[:, :, 2:4],
                                     _bcast(rcp[:, :, 0:1], 2))
                nc.vector.tensor_mul(out_sb[:, :, C_MB:C_MB + 54],
                                     ut_ps[:, :, 4:58],
                                     _bcast(rcp[:, :, 1:2], 54))
                _eng_t = nc.gpsimd if USE_GPSIMD_MBD else nc.vector
                t2 = small.tile([128, NSUB, D], F32)
                _eng_t.tensor_mul(t2[:],
                                  out_sb[:, :, C_MB + D:C_MB + 2 * D],
                                  _bcast(out_sb[:, :, C_MC + 1:C_MC + 2], D))
                t1 = small.tile([128, NSUB, D], F32)
                _eng_t.tensor_mul(t1[:],
                                  out_sb[:, :, C_MB:C_MB + D],
                                  _bcast(out_sb[:, :, C_MC:C_MC + 1], D))
                _eng_t.tensor_add(out_sb[:, :, C_MBD:C_MBD + D], t1[:], t2[:])

            def p3_transpose(it):
                st = state[it]
                out_sb = st["out"]
                mlpt_ps = ps_mlp.tile([D + 3, QT], F32)
                for s in range(NSUB):
                    nc.tensor.transpose(mlpt_ps[:, s * 128:(s + 1) * 128],
                                        out_sb[:, s, C_MBD:C_ALL],
                                        id_sb[:])
                st["mlpt_ps"] = mlpt_ps

            def p4_copy(it):
                st = state[it]
                mlpt_ps = st["mlpt_ps"]
                mlpt_sb = mlp.tile([D + 3, QT], F32)
                nc.vector.tensor_copy(mlpt_sb[:, 0:QT // 2],
                                      mlpt_ps[:, 0:QT // 2])
                nc.scalar.copy(mlpt_sb[:, QT // 2:QT],
                               mlpt_ps[:, QT // 2:QT])
                st["mlpt_sb"] = mlpt_sb

            def p5_mm3(it):
                st = state[it]
                h_ps = ps_h.tile([H, QT], F32)
                nc.tensor.matmul(h_ps[:], w1_sb[:], st["mlpt_sb"][:],
                                 start=True, stop=True)
                st["h_ps"] = h_ps

            def p6_relu(it):
                st = state[it]
                h_ps = st["h_ps"]
                h_sb = mlp.tile([H, QT], F32)
                nc.vector.tensor_scalar(out=h_sb[:, 0:QT // 2],
                                        in0=h_ps[:, 0:QT // 2],
                                        scalar1=b1_sb[:],
                                        scalar2=0.0,
                                        op0=mybir.AluOpType.add,
                                        op1=mybir.AluOpType.max)
                nc.scalar.activation(out=h_sb[:, QT // 2:QT],
                                     in_=h_ps[:, QT // 2:QT],
                                     func=mybir.ActivationFunctionType.Relu,
                                     bias=b1_sb[:], scale=1.0)
                st["h_sb"] = h_sb

            def p7_mm4(it):
                st = state[it]
                h_sb = st["h_sb"]
                rec_ps = ps_rec.tile([128, NSUB, D], F32)
                for s in range(NSUB):
                    nc.tensor.matmul(rec_ps[:, s, :],
                                     h_sb[:, s * 128:(s + 1) * 128],
                                     w2_sb[:], start=True, stop=True)
                st["rec_ps"] = rec_ps

            def p8_store(it):
                st = state.pop(it)
                out_sb = st["out"]
                b2b = bass.AP(tensor=b2_sb.tensor, offset=b2_sb.offset,
                              ap=[b2_sb.ap[0], [0, NSUB], b2_sb.ap[1]])
                nc.vector.tensor_add(out_sb[:, :, C_REC:C_REC + D],
                                     st["rec_ps"][:], b2b)
                o = it * QT
                dst = out_d[o:o + QT].rearrange("(s p) d -> p s d", p=128)
                nc.sync.dma_start(out=dst, in_=out_sb[:, :, 0:C_OUT])

            if K_STAGE >= 4:
                phases = [(0, p0_load), (1, p1_front), (2, p2_epilogue),
                          (3, p3_transpose), (3, p4_copy), (4, p5_mm3),
                          (4, p6_relu), (5, p7_mm4), (6, p8_store)]
            elif K_STAGE == 1:
                def p_dummy(it):
                    st = state[it]
                    nc.vector.memset(st["out"][:, :, 0:C_OUT], 0.0)

                def p_store1(it):
                    st = state.pop(it)
                    o = it * QT
                    dst = out_d[o:o + QT].rearrange("(s p) d -> p s d", p=128)
                    nc.sync.dma_start(out=dst, in_=st["out"][:, :, 0:C_OUT])
                phases = [(0, p0_load), (1, p_dummy), (2, p_store1)]
            else:
                raise ValueError("K_STAGE 2/3 removed in pipelined version")

            max_off = max(off for off, _ in phases)
            with rep_ctx:
                for step in range(NT + max_off):
                    for off, fn in phases:
                        it = step - off
                        if 0 <= it < NT:
                            fn(it)

    _split_excess_waits(nc)
    return nc


_PROGRAM = None


def _program():
    global _PROGRAM
    if _PROGRAM is None:
        _PROGRAM = _build_program()
    return _PROGRAM


# --------------------------------------------------------------------------
# Entry point
# --------------------------------------------------------------------------

def kernel(coords, coeff_mu, coeff_log_s, coeff_q, basis_mu, basis_log_s,
           basis_q, C, B, W1, b1, W2, b2):
    global LAST_EXEC_NS
    import os

    coords = np.asarray(coords, np.float32)

    A = np.concatenate([
        _quad_coeffs(np.asarray(coeff_mu), np.asarray(coeff_log_s), np.asarray(coeff_q)),
        _quad_coeffs(np.asarray(basis_mu), np.asarray(basis_log_s), np.asarray(basis_q)),
    ], axis=1).astype(np.float32)                    # [10, 128]
    a_hi, a_lo = _split_f32r(A)
    # mm1a pairs A_hi with [F_hi; F_lo]; mm1b pairs A_lo with F_hi.
    a1 = np.concatenate([a_hi, a_hi[0:9]], axis=0)   # [19, 128]
    a2 = a_lo                                        # [10, 128]

    CB = np.zeros((KG, 58), np.float32)
    CB[:MG, 0] = 1.0
    CB[MG:, 1] = 1.0
    CB[:MG, 2:4] = np.asarray(C, np.float32)
    CB[MG:, 4:58] = np.asarray(B, np.float32).reshape(NG, L * D)

    w1 = np.asarray(W1, np.float32)                  # [30, 32]
    w2 = np.asarray(W2, np.float32)                  # [32, 27]
    b1v = np.asarray(b1, np.float32).reshape(H, 1)
    b2b = np.broadcast_to(np.asarray(b2, np.float32).reshape(1, D), (128, D)).copy()
    ident = np.eye(128, dtype=np.float32)

    in_maps = []
    for c in range(NCORES):
        shard = coords[c * QC:(c + 1) * QC]
        f = _features_t(shard)                                             # [10, QC]
        f_hi, f_lo = _split_f32r(f)
        ft = np.ascontiguousarray(np.concatenate([f_hi, f_lo[0:9]], axis=0))  # [19, QC]
        co = np.ascontiguousarray(
            shard.reshape(NT, NSUB, 128, 3).transpose(0, 2, 1, 3))         # [NT,128,4,3]
        in_maps.append({
            "ft": ft, "co": co, "a1": a1, "a2": a2, "cb": CB, "w1": w1,
            "w2": w2, "b1": b1v, "b2b": b2b, "ident": ident,
        })

    global _LAST_IN_MAPS
    _LAST_IN_MAPS = in_maps
    nc = _program()
    trace = bool(int(os.environ.get("KERNEL_TRACE", "0")))
    res = run_bass_kernel_spmd(nc, in_maps, core_ids=list(range(NCORES)),
                               trace=trace)
    LAST_EXEC_NS = res.exec_time_ns

    out = np.concatenate([res.results[c]["out"] for c in range(NCORES)], axis=0)
    rec = np.ascontiguousarray(out[:, C_REC:C_REC + D])
    mc = np.ascontiguousarray(out[:, C_MC:C_MC + 2])
    mb = np.ascontiguousarray(out[:, C_MB:C_MB + 54]).reshape(Q, L, D)
    mbd = np.ascontiguousarray(out[:, C_MBD:C_MBD + D])
    return rec, mc, mb, mbd
